# revision 5
# baseline (speedup 1.0000x reference)
"""Trainium2 Bass kernel for group-quant (fake int8, V=64) + Linear.

reference math (per row of x):
    absmax over feature-groups of 64 -> delta = max(2*absmax/254, 1e-5)
    xq = clip(round(x/delta), -127, 127) * delta      (fake quant)
    out = xq @ W.T + b

Sharding: data-parallel on tokens across 8 cores (1024 rows each);
W (pre-transposed to [in,out] and cast fp16 on host) + b replicated.

Device pipeline per core (t-tile = 128 token rows, 8 per core):
  per t-tile: load x -> group absmax (vector) -> y=x*recip in place
  (gpsimd) -> q=round(y) via +/-1.5*2^23 magic, fp16 out (vector) ->
  xq=q*delta in place (gpsimd) -> 32 SBUF->SBUF XBAR DMA-transposes
  (scalar queue, transpose-only to avoid xbar_mode transitions) into a
  per-t x~^T tile.  Matmuls run oc-pass-outer so W streams from HBM
  exactly once; the first two oc-passes are interleaved over t so the
  PE has 2 psum-groups of work per quant-delivered tile during fill.
"""

import numpy as np

import concourse.bass as bass
import concourse.mybir as mybir
import concourse.tile as tile
from concourse.bass_utils import run_bass_kernel_spmd

N_CORES = 8
MAGIC = 1.5 * 2.0**23      # fp32 round-to-nearest-even constant
QSCALE = 1.0 / 127.0       # 2/(qmax-qmin) with qmax=127, qmin=-127
DELTA_MIN = 1e-5


def _split_multiwait(nc):
    """This walrus build allows at most ONE sync wait per instruction
    ("Too many sync wait commands", CoreV3GenImpl setupSyncWait) and none
    on Drain. Tile freely attaches several waits to one instruction, so
    post-process: move excess waits onto single-wait NoOps inserted just
    before the instruction on the same engine queue (semantics identical —
    the queue stalls at the nop instead of at the instruction)."""
    nid = 0
    for fn in nc.m.functions:
        for bb in fn.blocks:
            insts = list(bb.instructions)
            out = []
            changed = False
            for inst in insts:
                si = inst.sync_info
                waits = list(si.on_wait) if si is not None and si.on_wait else []
                limit = 0 if type(inst).__name__ == "InstDrain" else 1
                if len(waits) > limit:
                    changed = True
                    keep = waits[len(waits) - limit :] if limit else []
                    for w in waits[: len(waits) - limit]:
                        nid += 1
                        out.append(
                            mybir.InstNoOp(
                                name=f"WSPLIT-{nid}",
                                engine=inst.engine,
                                bass_nofuse=True,
                                ins=[],
                                outs=[],
                                sync_info=mybir.SyncInfo(on_wait=[w], on_update=[]),
                            )
                        )
                    si.on_wait = keep
                out.append(inst)
            if changed:
                try:
                    bb.instructions = out
                except Exception:
                    bb.instructions[:] = out


def build(T=1024, K=4096, O=4096, V=64, OC=512, split=True,
          ilv=2, wbufs=10, gps_quant=True):
    f32, f16 = mybir.dt.float32, mybir.dt.float16
    P = 128
    G = K // V                 # quant groups per row
    KT = K // P                # contraction tiles
    NOC = O // OC              # output chunks
    QW = 4                     # W-load quarters per o-chunk
    KQ = KT // QW
    NT = T // P                # token tiles per core
    assert NT * P == T

    nc = bass.Bass()
    x = nc.dram_tensor("x", [T, K], f32, kind="ExternalInput")
    wt = nc.dram_tensor("wt", [NOC, P, KT * OC], f16, kind="ExternalInput")
    bvec = nc.dram_tensor("b", [O], f32, kind="ExternalInput")
    out = nc.dram_tensor("out", [T, O], f32, kind="ExternalOutput")

    mult = mybir.AluOpType.mult
    add = mybir.AluOpType.add
    sub = mybir.AluOpType.subtract
    amax_op = mybir.AluOpType.max

    # quant compute engine for the two broadcast-multiply passes
    def qeng():
        return nc.gpsimd if gps_quant else nc.vector

    with tile.TileContext(nc) as tc:
        with (
            tc.tile_pool(name="xq", bufs=2) as pool_x,
            tc.tile_pool(name="xh", bufs=2) as pool_xh,
            tc.tile_pool(name="st", bufs=4) as pool_s,
            tc.tile_pool(name="xt", bufs=1) as pool_xt,
            tc.tile_pool(name="w", bufs=wbufs) as pool_w,
            tc.tile_pool(name="bias", bufs=3) as pool_b,
            tc.tile_pool(name="o", bufs=3) as pool_o,
            tc.tile_pool(name="ps", bufs=8, space="PSUM") as pool_ps,
        ):
            xT = [None] * NT           # per-t transposed tiles [P, KT, P]
            wq = {}                    # (oc, q) -> W quarter tile

            def load_w_quarter(oc, q):
                wqt = pool_w.tile([P, KQ, OC], f16, tag="w", name=f"w{oc}_{q}")
                nc.sync.dma_start(
                    out=wqt.rearrange("p kq o -> p (kq o)"),
                    in_=wt[oc][:, q * KQ * OC : (q + 1) * KQ * OC],
                )
                wq[(oc, q)] = wqt

            def load_bias(oc):
                btile = pool_b.tile([P, OC], f32, tag="bias", name=f"b{oc}")
                bsl = bvec[oc * OC : (oc + 1) * OC]
                b_bcast = bass.AP(
                    tensor=bsl.tensor, offset=bsl.offset, ap=[[0, P], *bsl.ap]
                )
                nc.sync.dma_start(out=btile[:], in_=b_bcast)
                return btile

            def quant_tile(t):
                """load + fake-quant t-tile, leave fp16 xq^T in xT[t]."""
                r0 = t * P
                xt_ = pool_x.tile([P, K], f32, tag="xq")
                nc.sync.dma_start(out=xt_[:], in_=x[r0 : r0 + P, :])
                x3 = xt_.rearrange("p (g v) -> p g v", v=V)
                amax = pool_s.tile([P, G], f32, tag="amax")
                nc.vector.tensor_reduce(
                    out=amax[:], in_=x3, axis=mybir.AxisListType.X,
                    op=amax_op, apply_absolute_value=True,
                )
                delta = pool_s.tile([P, G], f32, tag="delta")
                nc.vector.tensor_scalar(
                    out=delta[:], in0=amax[:],
                    scalar1=QSCALE, scalar2=DELTA_MIN, op0=mult, op1=amax_op,
                )
                recip = pool_s.tile([P, G], f32, tag="recip")
                nc.vector.reciprocal(out=recip[:], in_=delta[:])
                # y = x / delta, in place (broadcast recip over groups of V)
                qeng().tensor_tensor(
                    out=x3, in0=x3,
                    in1=recip[:, :, None].to_broadcast((P, G, V)), op=mult,
                )
                # q = round(y): exact fp32 RNE via +/-MAGIC; |y| <= 127 so the
                # integer result is exact in fp16
                xh_t = pool_xh.tile([P, K], f16, tag="xh")
                nc.vector.tensor_scalar(
                    out=xh_t[:], in0=xt_[:],
                    scalar1=MAGIC, scalar2=MAGIC, op0=add, op1=sub,
                )
                # xq = q * delta, in place fp16
                xh3 = xh_t.rearrange("p (g v) -> p g v", v=V)
                qeng().tensor_tensor(
                    out=xh3, in0=xh3,
                    in1=delta[:, :, None].to_broadcast((P, G, V)), op=mult,
                )
                # SBUF->SBUF XBAR transposes, one per k-tile, scalar queue only
                xT[t] = pool_xt.tile([P, KT, P], f16, tag=f"xT{t}", name=f"xT{t}")
                for k in range(KT):
                    nc.scalar.dma_start_transpose(
                        xT[t][:, k, :], xh_t[:, k * P : (k + 1) * P]
                    )

            def mm_group(oc, t, btile):
                ps = pool_ps.tile([P, OC], f32, tag="ps")
                for k in range(KT):
                    nc.tensor.matmul(
                        ps[:],
                        xT[t][:, k, :],
                        wq[(oc, k // KQ)][:, k % KQ, :],
                        start=(k == 0),
                        stop=(k == KT - 1),
                    )
                ot = pool_o.tile([P, OC], f32, tag="o")
                nc.vector.tensor_tensor(out=ot[:], in0=ps[:], in1=btile[:], op=add)
                nc.sync.dma_start(
                    out=out[t * P : (t + 1) * P, oc * OC : (oc + 1) * OC], in_=ot[:]
                )

            # ---- emission order ----
            # sync queue: x0, W(first ilv passes) quarters, x1.., biases
            quant_tile(0)
            for q in range(QW):
                for oc in range(ilv):
                    load_w_quarter(oc, q)
            bt = {oc: load_bias(oc) for oc in range(ilv)}
            for t in range(1, NT):
                quant_tile(t)
            # fill phase: first ilv oc-passes interleaved over t
            for t in range(NT):
                for oc in range(ilv):
                    mm_group(oc, t, bt[oc])
            # steady state: remaining oc-passes, W streamed once
            for oc in range(ilv, NOC):
                for q in range(QW):
                    load_w_quarter(oc, q)
                btile = load_bias(oc)
                for t in range(NT):
                    mm_group(oc, t, btile)
    if split:
        _split_multiwait(nc)
    return nc


_CACHED = {}

# test-harness knobs (kernel() defaults are what the grader uses)
TRACE = False
LAST_RESULT = None
BUILD_KW = {}


def _get_nc(shape_key):
    key = (shape_key, tuple(sorted(BUILD_KW.items())))
    if key not in _CACHED:
        T, K, O = shape_key
        _CACHED[key] = build(T=T, K=K, O=O, **BUILD_KW)
    return _CACHED[key]


def pack_w(W: np.ndarray, OC: int = 512, P: int = 128) -> np.ndarray:
    # [out,in] -> W^T [in,out] fp16, packed [NOC, P, KT*OC] so each per-core
    # o-chunk W load is one fully contiguous DMA
    K, O = W.shape[1], W.shape[0]
    KT, NOC = K // P, O // OC
    wt = np.ascontiguousarray(W.T).astype(np.float16)         # [K, O]
    z = wt.reshape(KT, P, NOC, OC).transpose(2, 1, 0, 3)      # [NOC, P, KT, OC]
    return np.ascontiguousarray(z.reshape(NOC, P, KT * OC))


def kernel(x: np.ndarray, W: np.ndarray, b: np.ndarray) -> np.ndarray:
    global LAST_RESULT
    n, k = x.shape               # 8192, 4096
    o = W.shape[0]               # 4096
    assert n % N_CORES == 0
    tpc = n // N_CORES
    nc = _get_nc((tpc, k, o))

    wt = pack_w(W)
    b32 = np.ascontiguousarray(b.astype(np.float32))
    xs = np.ascontiguousarray(x.astype(np.float32)).reshape(N_CORES, tpc, k)
    in_maps = [{"x": xs[i], "wt": wt, "b": b32} for i in range(N_CORES)]
    res = run_bass_kernel_spmd(nc, in_maps, list(range(N_CORES)), trace=TRACE)
    LAST_RESULT = res
    return np.concatenate([res.results[i]["out"] for i in range(N_CORES)], axis=0)


# revision 7
# speedup vs baseline: 1.2084x; 1.2084x over previous
"""Trainium2 Bass kernel for group-quant (fake int8, V=64) + Linear.

reference math (per row of x):
    absmax over feature-groups of 64 -> delta = max(2*absmax/254, 1e-5)
    xq = clip(round(x/delta), -127, 127) * delta      (fake quant)
    out = xq @ W.T + b

Sharding: data-parallel on tokens across 8 cores (1024 rows each);
W (pre-transposed to [in,out] and cast fp16 on host) + b replicated.

Device pipeline per core (t-tile = 128 token rows, 8 per core):
  per t-tile: load x -> group absmax (vector) -> y=x*recip in place
  (gpsimd) -> q=round(y) via +/-1.5*2^23 magic, fp16 out (vector) ->
  xq=q*delta in place (vector) -> store fp16 to a per-pair DRAM bounce
  -> per PAIR of t-tiles, 32 XBAR DMA-transposes [256,128] split over
  the two HWDGE queues (sync+scalar, transpose-only so xbar_mode never
  transitions).  All copy DMAs (x in, xh out, W, bias, out) ride the
  gpsimd SWDGE queue.  Matmuls run oc-pass-outer so W streams from HBM
  exactly once (as [128,4,512] eighths for fine-grained prefetch); the
  first two oc-passes are interleaved over t so the PE has two
  psum-groups of work per quant-delivered t-tile during fill.
"""

import numpy as np

import concourse.bass as bass
import concourse.mybir as mybir
import concourse.tile as tile
from concourse.bass_utils import run_bass_kernel_spmd

N_CORES = 8
MAGIC = 1.5 * 2.0**23      # fp32 round-to-nearest-even constant
QSCALE = 1.0 / 127.0       # 2/(qmax-qmin) with qmax=127, qmin=-127
DELTA_MIN = 1e-5


def _split_multiwait(nc):
    """This walrus build allows at most ONE sync wait per instruction
    ("Too many sync wait commands", CoreV3GenImpl setupSyncWait) and none
    on Drain. Tile freely attaches several waits to one instruction, so
    post-process: move excess waits onto single-wait NoOps inserted just
    before the instruction on the same engine queue (semantics identical —
    the queue stalls at the nop instead of at the instruction)."""
    nid = 0
    for fn in nc.m.functions:
        for bb in fn.blocks:
            insts = list(bb.instructions)
            out = []
            changed = False
            for inst in insts:
                si = inst.sync_info
                waits = list(si.on_wait) if si is not None and si.on_wait else []
                limit = 0 if type(inst).__name__ == "InstDrain" else 1
                if len(waits) > limit:
                    changed = True
                    keep = waits[len(waits) - limit :] if limit else []
                    for w in waits[: len(waits) - limit]:
                        nid += 1
                        out.append(
                            mybir.InstNoOp(
                                name=f"WSPLIT-{nid}",
                                engine=inst.engine,
                                bass_nofuse=True,
                                ins=[],
                                outs=[],
                                sync_info=mybir.SyncInfo(on_wait=[w], on_update=[]),
                            )
                        )
                    si.on_wait = keep
                out.append(inst)
            if changed:
                try:
                    bb.instructions = out
                except Exception:
                    bb.instructions[:] = out


def build(T=1024, K=4096, O=4096, V=64, OC=512, split=True,
          ilv=2, wbufs=17, WE=8):
    f32, f16 = mybir.dt.float32, mybir.dt.float16
    P = 128
    G = K // V                 # quant groups per row
    KT = K // P                # contraction tiles
    NOC = O // OC              # output chunks
    KE = KT // WE              # k-tiles per W-load eighth
    NT = T // P                # token tiles per core
    NP = NT // 2               # t-tile pairs
    assert NT * P == T

    nc = bass.Bass()
    x = nc.dram_tensor("x", [T, K], f32, kind="ExternalInput")
    wt = nc.dram_tensor("wt", [NOC, P, KT * OC], f16, kind="ExternalInput")
    bvec = nc.dram_tensor("b", [O], f32, kind="ExternalInput")
    out = nc.dram_tensor("out", [T, O], f32, kind="ExternalOutput")
    # per-pair DRAM bounce for quantized fp16 x (disjoint tensors keep the
    # store->transpose dependencies precise)
    xhp = [nc.dram_tensor(f"xhp{p}", [2 * P, K], f16) for p in range(NP)]

    mult = mybir.AluOpType.mult
    add = mybir.AluOpType.add
    sub = mybir.AluOpType.subtract
    amax_op = mybir.AluOpType.max

    with tile.TileContext(nc) as tc:
        with (
            tc.tile_pool(name="xq", bufs=2) as pool_x,
            tc.tile_pool(name="xh", bufs=2) as pool_xh,
            tc.tile_pool(name="st", bufs=3) as pool_s,
            tc.tile_pool(name="xt", bufs=1) as pool_xt,
            tc.tile_pool(name="w", bufs=wbufs) as pool_w,
            tc.tile_pool(name="bias", bufs=1) as pool_b,
            tc.tile_pool(name="o", bufs=2) as pool_o,
            tc.tile_pool(name="ps", bufs=8, space="PSUM") as pool_ps,
        ):
            xT = [None] * NP           # per-pair transposed tiles [P, KT, 2P]
            wq = {}                    # (oc, e) -> W eighth tile [P, KE, OC]

            def load_w_eighth(oc, e):
                wqt = pool_w.tile([P, KE, OC], f16, tag="w", name=f"w{oc}_{e}")
                nc.gpsimd.dma_start(
                    out=wqt.rearrange("p ke o -> p (ke o)"),
                    in_=wt[oc][:, e * KE * OC : (e + 1) * KE * OC],
                )
                wq[(oc, e)] = wqt

            def quant_load(t):
                xt_ = pool_x.tile([P, K], f32, tag="xq", name=f"x{t}")
                nc.gpsimd.dma_start(out=xt_[:], in_=x[t * P : (t + 1) * P, :])
                return xt_

            def quant_rest(t, xt_):
                x3 = xt_.rearrange("p (g v) -> p g v", v=V)
                amax = pool_s.tile([P, G], f32, tag="amax")
                nc.vector.tensor_reduce(
                    out=amax[:], in_=x3, axis=mybir.AxisListType.X,
                    op=amax_op, apply_absolute_value=True,
                )
                delta = pool_s.tile([P, G], f32, tag="delta")
                nc.vector.tensor_scalar(
                    out=delta[:], in0=amax[:],
                    scalar1=QSCALE, scalar2=DELTA_MIN, op0=mult, op1=amax_op,
                )
                recip = pool_s.tile([P, G], f32, tag="recip")
                nc.vector.reciprocal(out=recip[:], in_=delta[:])
                # y = x / delta, in place (broadcast recip over groups of V)
                nc.gpsimd.tensor_tensor(
                    out=x3, in0=x3,
                    in1=recip[:, :, None].to_broadcast((P, G, V)), op=mult,
                )
                # q = round(y): exact fp32 RNE via +/-MAGIC; |y| <= 127 so the
                # integer result is exact in fp16
                xh_t = pool_xh.tile([P, K], f16, tag="xh")
                nc.vector.tensor_scalar(
                    out=xh_t[:], in0=xt_[:],
                    scalar1=MAGIC, scalar2=MAGIC, op0=add, op1=sub,
                )
                # xq = q * delta, in place fp16
                xh3 = xh_t.rearrange("p (g v) -> p g v", v=V)
                nc.vector.tensor_tensor(
                    out=xh3, in0=xh3,
                    in1=delta[:, :, None].to_broadcast((P, G, V)), op=mult,
                )
                nc.gpsimd.dma_start(
                    out=xhp[t // 2][(t % 2) * P : (t % 2 + 1) * P, :], in_=xh_t[:]
                )

            def transpose_pair(p):
                # [256,128] DRAM -> [128,256] SBUF per k-tile, split over both
                # HWDGE queues (transpose-only traffic on each)
                xT[p] = pool_xt.tile([P, KT, 2 * P], f16, tag=f"xT{p}", name=f"xT{p}")
                for k in range(KT):
                    eng = nc.scalar if k % 2 == 0 else nc.sync
                    eng.dma_start_transpose(
                        xT[p][:, k, :], xhp[p][:, k * P : (k + 1) * P]
                    )

            def mm_group(oc, t, btile):
                ps = pool_ps.tile([P, OC], f32, tag="ps")
                for k in range(KT):
                    nc.tensor.matmul(
                        ps[:],
                        xT[t // 2][:, k, (t % 2) * P : (t % 2 + 1) * P],
                        wq[(oc, k // KE)][:, k % KE, :],
                        start=(k == 0),
                        stop=(k == KT - 1),
                    )
                ot = pool_o.tile([P, OC], f32, tag="o")
                nc.vector.tensor_tensor(
                    out=ot[:], in0=ps[:],
                    in1=btile[:, oc * OC : (oc + 1) * OC], op=add,
                )
                nc.gpsimd.dma_start(
                    out=out[t * P : (t + 1) * P, oc * OC : (oc + 1) * OC], in_=ot[:]
                )

            # ---- emission ----
            x0 = quant_load(0)
            x1 = quant_load(1)
            # W for the first ilv passes + full bias right behind x0/x1
            for e in range(WE):
                for oc in range(ilv):
                    load_w_eighth(oc, e)
            btile = pool_b.tile([P, O], f32, tag="bias", name="bias")
            bsl = bvec[0:O]
            b_bcast = bass.AP(tensor=bsl.tensor, offset=bsl.offset,
                              ap=[[0, P], *bsl.ap])
            nc.gpsimd.dma_start(out=btile[:], in_=b_bcast)
            quant_rest(0, x0)
            quant_rest(1, x1)
            transpose_pair(0)
            for t in range(2, NT):
                xt_ = quant_load(t)
                quant_rest(t, xt_)
                if t % 2 == 1:
                    transpose_pair(t // 2)
            # fill: first ilv oc-passes interleaved over t, with W prefetch
            # for the following passes interleaved between groups
            pf = [(oc, e) for oc in range(ilv, NOC) for e in range(WE)]
            pfi = 0
            for t in range(NT):
                for oc in range(ilv):
                    mm_group(oc, t, btile)
                npf = 6 if t > 0 else 0
                for _ in range(npf):
                    if pfi < len(pf):
                        load_w_eighth(*pf[pfi]); pfi += 1
            # steady state: remaining oc-passes
            for oc in range(ilv, NOC):
                for t in range(NT):
                    mm_group(oc, t, btile)
                    if pfi < len(pf):
                        load_w_eighth(*pf[pfi]); pfi += 1
    if split:
        _split_multiwait(nc)
    return nc


_CACHED = {}

# test-harness knobs (kernel() defaults are what the grader uses)
TRACE = False
LAST_RESULT = None
BUILD_KW = {}


def _get_nc(shape_key):
    key = (shape_key, tuple(sorted(BUILD_KW.items())))
    if key not in _CACHED:
        T, K, O = shape_key
        _CACHED[key] = build(T=T, K=K, O=O, **BUILD_KW)
    return _CACHED[key]


def pack_w(W: np.ndarray, OC: int = 512, P: int = 128) -> np.ndarray:
    # [out,in] -> W^T [in,out] fp16, packed [NOC, P, KT*OC] so each per-core
    # o-chunk W load is one fully contiguous DMA
    K, O = W.shape[1], W.shape[0]
    KT, NOC = K // P, O // OC
    wt = np.ascontiguousarray(W.T).astype(np.float16)         # [K, O]
    z = wt.reshape(KT, P, NOC, OC).transpose(2, 1, 0, 3)      # [NOC, P, KT, OC]
    return np.ascontiguousarray(z.reshape(NOC, P, KT * OC))


def kernel(x: np.ndarray, W: np.ndarray, b: np.ndarray) -> np.ndarray:
    global LAST_RESULT
    n, k = x.shape               # 8192, 4096
    o = W.shape[0]               # 4096
    assert n % N_CORES == 0
    tpc = n // N_CORES
    nc = _get_nc((tpc, k, o))

    wt = pack_w(W)
    b32 = np.ascontiguousarray(b.astype(np.float32))
    xs = np.ascontiguousarray(x.astype(np.float32)).reshape(N_CORES, tpc, k)
    in_maps = [{"x": xs[i], "wt": wt, "b": b32} for i in range(N_CORES)]
    res = run_bass_kernel_spmd(nc, in_maps, list(range(N_CORES)), trace=TRACE)
    LAST_RESULT = res
    return np.concatenate([res.results[i]["out"] for i in range(N_CORES)], axis=0)
